# revision 20
# baseline (speedup 1.0000x reference)
"""Trainium2 Bass kernel for nn_Attention_54391465836966.

The reference's .reshape calls are RAW byte reinterpretations: token matrix
T = content_feat[b] bytes viewed [S, C] (not a transpose), and s (token-major
[S, C]) is viewed [C, S] before the 1x1 conv.  The host passes the token
views pre-transposed to channel-major (cfT/compT/posT) so the device does no
PE transposes; the s view is realized with SBUF->SBUF DMAs that re-pair
token rows (s2d[r] = tokens (2r, 2r+1) concatenated).

Per core (b = core//4, n = core%4), channel-major [C, S] throughout:
  ctok = cfT + posT ; ctmp = compT + posT
  qT = Wq^T ctok ; kT = Wkv[:, :C]^T ctmp ; v = ctmp^T Wkv[:, C:]
  per head h: P = exp(scale k_h^T q); o_h = (v_h^T P) / Z   (Z via ones col)
  s_tok = packed^T Wproj + ctok^T (Wproj/4) + bproj/4       (token-major)
  out_p = WconvT[:C]^T s2d + bconv/4 ; out_cf = WconvT[C:, quarter]^T cf_raw
Host sums the 4 component partials per batch and places the out_cf quarter
rows.  The affine const terms ride every core at 1/4 scale so no gated-zero
work exists; heads are packed two-per-tile so proj runs at 128 contraction.
P/V run in bf16 (PE rate unchanged, half the SBUF traffic); other matmuls
fp32r.
"""
import sys

sys.path.insert(0, "/opt/trn_rl_repo")

import numpy as np

N_CORES = 8
B, C, H, W = 2, 512, 32, 32
S = H * W  # 1024
NH, HD = 8, 64
SCALE = HD ** -0.5

_CACHE = {}


def _build():
    if "nc" in _CACHE:
        return _CACHE["nc"]
    from contextlib import ExitStack

    import concourse.bacc as bacc
    import concourse.mybir as mybir
    import concourse.tile as tile

    f32 = mybir.dt.float32
    f32r = mybir.dt.float32r
    bf16 = mybir.dt.bfloat16
    EXP = mybir.ActivationFunctionType.Exp

    nc = bacc.Bacc("TRN2", target_bir_lowering=False, debug=False,
                   num_devices=N_CORES)

    din = lambda n, s, dt: nc.dram_tensor(n, s, dt, kind="ExternalInput").ap()
    cfT = din("cfT", [C, S], f32r)      # token view of content_feat[b], c-major
    cfr = din("cfr", [C, S], f32r)      # raw content_feat[b] (conv input)
    compT = din("compT", [C, S], f32r)  # token view of components[n, b]
    posT = din("posT", [C, S], f32r)    # pos_emb[0].T
    wq = din("wq", [C, C], f32r)
    wkv = din("wkv", [C, 2 * C], f32r)
    wproj = din("wproj", [C, C], f32r)
    wproj4 = din("wproj4", [C, C], f32r)     # Wproj / 4
    bproj4 = din("bproj4", [1, C], f32r)     # bproj / 4
    wcvs = din("wcvs", [C, C], f32r)         # WconvT[:C]
    wcvcq = din("wcvcq", [C, 128], f32r)     # WconvT[C:, 128n:128(n+1)]
    bconv4 = din("bconv4", [1, C], f32r)     # bconv / 4
    out_p = nc.dram_tensor("out_p", [C, S], f32, kind="ExternalOutput").ap()
    out_cf = nc.dram_tensor("out_cf", [128, S], f32,
                            kind="ExternalOutput").ap()

    with tile.TileContext(nc) as tc, ExitStack() as ctx:
        main = ctx.enter_context(tc.tile_pool(name="main", bufs=1))
        trans = ctx.enter_context(tc.tile_pool(name="trans", bufs=2))

        # ---- constants ----
        ones32 = main.tile([1, 512], f32, tag="ones32")
        nc.gpsimd.memset(ones32[:], 1.0)
        ones = main.tile([1, 512], f32r, tag="ones")
        nc.vector.tensor_copy(ones[:], ones32[:])

        # ---- token tiles: ctok (content) on DVE, comp_tok on Pool ----
        cf_sb = [main.tile([128, S], f32r, tag=f"cf{j}", name=f"cf{j}")
                 for j in range(4)]
        cfr_sb = [main.tile([128, S], f32r, tag=f"cfr{j}", name=f"cfr{j}")
                  for j in range(4)]
        ctok = [main.tile([128, S], f32r, tag=f"ctk{j}", name=f"ctok{j}")
                for j in range(4)]
        ctmp = [main.tile([128, S], f32r, tag=f"ct{j}", name=f"ctmp{j}")
                for j in range(4)]
        for j in range(4):
            pos_t = trans.tile([128, S], f32r, tag="pos", bufs=2)
            nc.sync.dma_start(pos_t[:], posT[128 * j:128 * (j + 1), :])
            nc.sync.dma_start(cf_sb[j][:], cfT[128 * j:128 * (j + 1), :])
            cmp_t = trans.tile([128, S], f32r, tag="cmp", bufs=2)
            nc.sync.dma_start(cmp_t[:], compT[128 * j:128 * (j + 1), :])
            nc.gpsimd.tensor_add(ctmp[j][:], cmp_t[:], pos_t[:])
            nc.vector.tensor_add(ctok[j][:], cf_sb[j][:], pos_t[:])

        # ---- weights ----
        wkv_sb = [main.tile([128, 2 * C], f32r, tag=f"wkv{k}", name=f"wkv{k}")
                  for k in range(4)]
        wq_sb = [main.tile([128, C], f32r, tag=f"wq{k}", name=f"wq{k}")
                 for k in range(4)]
        for k in range(4):
            nc.sync.dma_start(wkv_sb[k][:], wkv[128 * k:128 * (k + 1), :])
        for k in range(4):
            nc.sync.dma_start(wq_sb[k][:], wq[128 * k:128 * (k + 1), :])
        wcvcq_sb = [main.tile([128, 128], f32r, tag=f"wcc{k}", name=f"wcc{k}")
                    for k in range(4)]
        for k in range(4):
            nc.sync.dma_start(wcvcq_sb[k][:], wcvcq[128 * k:128 * (k + 1), :])
            nc.sync.dma_start(cfr_sb[k][:], cfr[128 * k:128 * (k + 1), :])
        wproj_sb = [main.tile([128, C], f32r, tag=f"wp{j}", name=f"wp{j}")
                    for j in range(4)]
        wproj4_sb = [main.tile([128, C], f32r, tag=f"wp4{k}", name=f"wp4{k}")
                     for k in range(4)]
        wcvs_sb = [main.tile([128, C], f32r, tag=f"wcs{k}", name=f"wcs{k}")
                   for k in range(4)]
        bproj4_sb = main.tile([1, C], f32r, tag="bp4")
        bconv4_sb = main.tile([1, C], f32r, tag="bc4")
        for k in range(4):
            nc.sync.dma_start(wproj_sb[k][:], wproj[128 * k:128 * (k + 1), :])
            nc.sync.dma_start(wproj4_sb[k][:], wproj4[128 * k:128 * (k + 1), :])
            nc.sync.dma_start(wcvs_sb[k][:], wcvs[128 * k:128 * (k + 1), :])
        nc.sync.dma_start(bproj4_sb[:], bproj4[:])
        nc.sync.dma_start(bconv4_sb[:], bconv4[:])

        # norm scratch rides the dead trans slots; packed rides wq slots
        zraw = trans.tile([1, S], f32, tag="pos", bufs=2, name="zraw")
        zs2 = trans.tile([1, S], f32, tag="pos", bufs=2, name="zs2")
        zinv = trans.tile([1, S], f32, tag="cmp", bufs=2, name="zinv")
        zbc = trans.tile([128, S], f32, tag="cmp", bufs=2, name="zbc")
        ocf_sb = main.tile([128, S], f32, tag="ocf")
        packed = [main.tile([128, S], f32r, tag=f"wq{j}", name=f"pk{j}")
                  for j in range(4)]
        outp = [main.tile([128, S], f32, tag=f"cf{j}", name=f"op{j}")
                for j in range(4)]

        with tc.tile_pool(name="psA", bufs=2, space="PSUM") as ps:
            # ---- out_cf: cf-conv output-channel quarter (independent) ----
            for half in range(2):
                acc = ps.tile([128, 512], f32, tag="mm")
                for k in range(4):
                    nc.tensor.matmul(
                        acc[:], wcvcq_sb[k][:],
                        cfr_sb[k][:, 512 * half:512 * (half + 1)],
                        start=(k == 0), stop=(k == 3))
                nc.scalar.copy(ocf_sb[:, 512 * half:512 * (half + 1)], acc[:])
            nc.sync.dma_start(out_cf[:, :], ocf_sb[:])

            # ---- v: [ki, hd] blocks per (kt, head) + ones col, bf16 ----
            v_sb = [main.tile([128, 1040], bf16, tag=f"v{t}", name=f"v{t}")
                    for t in range(4)]
            for t in range(4):
                nc.gpsimd.memset(v_sb[t][:], 1.0)
            for kt in range(8):
                acc = ps.tile([128, 512], f32, tag="mm")
                for k in range(4):
                    nc.tensor.matmul(acc[:],
                                     ctmp[k][:, 128 * kt:128 * (kt + 1)],
                                     wkv_sb[k][:, C:2 * C],
                                     start=(k == 0), stop=(k == 3))
                dst = v_sb[kt // 2][:, 520 * (kt % 2):520 * (kt % 2) + 520]
                nc.scalar.copy(
                    dst.rearrange("p (m c) -> p m c", m=8)[:, :, 0:64],
                    acc[:].rearrange("p (m c) -> p m c", m=8))

            # ---- kT / qT ----
            kT = [main.tile([128, S], f32r, tag=f"kT{j}", name=f"kT{j}")
                  for j in range(4)]
            qT = [main.tile([128, S], f32r, tag=f"qT{j}", name=f"qT{j}")
                  for j in range(4)]
            for j in range(4):
                for qc in range(2):
                    acc = ps.tile([128, 512], f32, tag="mm")
                    for k in range(4):
                        nc.tensor.matmul(acc[:],
                                         wkv_sb[k][:, 128 * j:128 * (j + 1)],
                                         ctmp[k][:, 512 * qc:512 * (qc + 1)],
                                         start=(k == 0), stop=(k == 3))
                    nc.vector.tensor_copy(kT[j][:, 512 * qc:512 * (qc + 1)],
                                          acc[:])
                    acc2 = ps.tile([128, 512], f32, tag="mm")
                    for k in range(4):
                        nc.tensor.matmul(acc2[:],
                                         wq_sb[k][:, 128 * j:128 * (j + 1)],
                                         ctok[k][:, 512 * qc:512 * (qc + 1)],
                                         start=(k == 0), stop=(k == 3))
                    nc.scalar.copy(qT[j][:, 512 * qc:512 * (qc + 1)],
                                   acc2[:])

        # ---- attention + tail in one PSUM pool set ----
        with tc.tile_pool(name="psS", bufs=2, space="PSUM") as psS, \
             tc.tile_pool(name="psO", bufs=1, space="PSUM") as psO, \
             tc.tile_pool(name="psT", bufs=2, space="PSUM") as psT:
            ptp = [main.tile([128, S], bf16, tag=f"pt{t}", name=f"pt{t}")
                   for t in range(8)]
            for h in range(NH):
                j, row = h // 2, 64 * (h % 2)
                o_ps = psO.tile([65, S], f32, tag="o")
                for kt in range(8):
                    sc = psS.tile([128, S], f32, tag="sc")
                    for qc in range(2):
                        nc.tensor.matmul(
                            sc[:, 512 * qc:512 * (qc + 1)],
                            kT[j][row:row + 64, 128 * kt:128 * (kt + 1)],
                            qT[j][row:row + 64, 512 * qc:512 * (qc + 1)],
                            start=True, stop=True)
                    pt = ptp[kt]
                    nc.scalar.activation(pt[:], sc[:], EXP, scale=SCALE)
                    vsl = v_sb[kt // 2][:, 520 * (kt % 2) + 65 * h:
                                        520 * (kt % 2) + 65 * h + 65]
                    for qc in range(2):
                        nc.tensor.matmul(
                            o_ps[:, 512 * qc:512 * (qc + 1)], vsl,
                            pt[:, 512 * qc:512 * (qc + 1)],
                            start=(kt == 0), stop=(kt == 7))
                # normalization: Z row -> recip -> broadcast -> scale
                nc.vector.tensor_copy(zraw[0:1, :], o_ps[64:65, :])
                nc.vector.reciprocal_approx_accurate(
                    zinv[0:1, :], zraw[0:1, :], zs2[0:1, :])
                nc.gpsimd.partition_broadcast(zbc[:], zinv[0:1, :])
                rows = slice(64 * (h % 2), 64 * (h % 2) + 64)
                nc.vector.tensor_copy(packed[j][rows, :], o_ps[0:64, :])
                nc.gpsimd.tensor_mul(packed[j][rows, :],
                                     packed[j][rows, :], zbc[rows, :])

            # ---- proj (token-major) -> s_sb -> s2d re-pair -> conv ----
            s_sb = [main.tile([128, 512], f32r, tag=f"qT{i % 4}",
                              name=f"s{i}") for i in range(8)]
            s2d = [main.tile([128, S], f32r, tag=f"kT{i}", name=f"s2d{i}")
                   for i in range(4)]
            for i in range(8):
                acc = psT.tile([128, 512], f32, tag="pj")
                nc.tensor.matmul(acc[:], ones[0:1, 0:128], bproj4_sb[:],
                                 start=True, stop=False)
                for k in range(4):
                    nc.tensor.matmul(acc[:],
                                     ctok[k][:, 128 * i:128 * (i + 1)],
                                     wproj4_sb[k][:], start=False, stop=False)
                for jj in range(4):
                    nc.tensor.matmul(acc[:],
                                     packed[jj][:, 128 * i:128 * (i + 1)],
                                     wproj_sb[jj][:], start=False,
                                     stop=(jj == 3))
                eng = nc.vector if i % 2 == 0 else nc.scalar
                if eng is nc.scalar:
                    eng.copy(s_sb[i][:], acc[:])
                else:
                    eng.tensor_copy(s_sb[i][:], acc[:])
                # re-pair tokens into the raw [C, S] view as each pair lands
                if i % 2 == 1:
                    ii = i // 2
                    for g in range(2):
                        for sh in range(2):  # source half by r_loc
                            src = s_sb[2 * ii + sh]
                            nc.sync.dma_start(
                                s2d[ii][64 * sh:64 * sh + 64,
                                        512 * g:512 * g + 512],
                                src[g:128:2, :])
            for oc in range(4):
                for half in range(2):
                    acc = psT.tile([128, 512], f32, tag="pj")
                    nc.tensor.matmul(acc[:],
                                     bconv4_sb[0:1, 128 * oc:128 * (oc + 1)],
                                     ones[0:1, :], start=True, stop=False)
                    for r in range(4):
                        nc.tensor.matmul(
                            acc[:], wcvs_sb[r][:, 128 * oc:128 * (oc + 1)],
                            s2d[r][:, 512 * half:512 * (half + 1)],
                            start=False, stop=(r == 3))
                    eng = nc.vector if (oc + half) % 2 == 0 else nc.scalar
                    if eng is nc.scalar:
                        eng.copy(outp[oc][:, 512 * half:512 * (half + 1)],
                                 acc[:])
                    else:
                        eng.tensor_copy(
                            outp[oc][:, 512 * half:512 * (half + 1)], acc[:])
                    nc.sync.dma_start(
                        out_p[128 * oc:128 * (oc + 1),
                              512 * half:512 * (half + 1)],
                        outp[oc][:, 512 * half:512 * (half + 1)])

    nc.compile()
    _CACHE["nc"] = nc
    return nc


def _shard_inputs(content_feat, components, pos_emb, Wq, Wkv, Wproj, bproj,
                  Wconv, bconv):
    f = np.float32
    posT = np.ascontiguousarray(pos_emb.reshape(S, C).T, dtype=f)
    wq2 = np.ascontiguousarray(Wq, dtype=f)
    wkv2 = np.ascontiguousarray(Wkv, dtype=f)
    wp2 = np.ascontiguousarray(Wproj, dtype=f)
    wp4 = np.ascontiguousarray(Wproj / 4, dtype=f)
    bp4 = np.ascontiguousarray((bproj / 4).reshape(1, C), dtype=f)
    wconvT = np.ascontiguousarray(Wconv.T, dtype=f)   # [2C, C]
    wcvs = np.ascontiguousarray(wconvT[:C], dtype=f)
    bc4 = np.ascontiguousarray((bconv / 4).reshape(1, C), dtype=f)
    in_maps = []
    for core in range(N_CORES):
        b, n = core // 4, core % 4
        sl = slice(128 * n, 128 * (n + 1))
        in_maps.append({
            "cfT": np.ascontiguousarray(content_feat[b].reshape(S, C).T,
                                        dtype=f),
            "cfr": np.ascontiguousarray(content_feat[b].reshape(C, S),
                                        dtype=f),
            "compT": np.ascontiguousarray(components[n, b].reshape(S, C).T,
                                          dtype=f),
            "posT": posT,
            "wq": wq2,
            "wkv": wkv2,
            "wproj": wp2,
            "wproj4": wp4,
            "bproj4": bp4,
            "wcvs": wcvs,
            "wcvcq": np.ascontiguousarray(wconvT[C:, sl], dtype=f),
            "bconv4": bc4,
        })
    return in_maps


def _run(trace=False, **inputs):
    from concourse.bass_utils import run_bass_kernel_spmd

    nc = _build()
    in_maps = _shard_inputs(**inputs)
    res = run_bass_kernel_spmd(nc, in_maps, list(range(N_CORES)), trace=trace)
    full = np.empty((B, C, S), dtype=np.float32)
    for b in range(B):
        acc = sum(res.results[4 * b + n]["out_p"] for n in range(4))
        for n in range(4):
            acc[128 * n:128 * (n + 1)] += res.results[4 * b + n]["out_cf"]
        full[b] = acc
    return full.reshape(B, C, H, W).astype(np.float32), res


def kernel(**inputs):
    out, _ = _run(trace=False, **inputs)
    return out
